# revision 1
# baseline (speedup 1.0000x reference)
"""Trainium2 Bass kernel for per-edge dot products (DGL u_dot_v / DotPredictor).

score[e] = sum_d h[src[e], d] * h[dst[e], d]

Strategy:
  - Split the E=6.4M edges evenly across 8 NeuronCores (800k each); replicate
    the node table h (100000x128 fp32, 51MB) in each core's HBM.
  - Bulk row gather uses the GPSIMD ucode `dma_gather` (InstDMAGatherAnt):
    thousands of 512B row fetches per instruction. Its indices are int16, so
    the node table is viewed as 4 segments of 25000 rows and each core's edges
    are bucketed on the host into 16 (src_seg, dst_seg) buckets (padded to a
    fixed size so the SPMD program is static). Edges past the pad (won't
    happen for the expected distribution) fall back to a host dot product.
  - Per 5120-edge chunk: gather h[src] and h[dst] rows to SBUF, multiply +
    per-row reduce on the vector engine, stream 1 score/edge back to HBM.
  - Host unpermutes scores back to the original edge order.
"""
import sys

sys.path.insert(0, "/opt/trn_rl_repo")

import numpy as np

import concourse.bacc as bacc
import concourse.bass as bass
import concourse.mybir as mybir
import concourse.tile as tile
from concourse.bass_utils import run_bass_kernel_spmd

# Problem shape (hardcoded per contract).
N, E, D = 100000, 6400000, 128
M = 8                      # NeuronCores
P = 128                    # SBUF partitions
E_PER = E // M             # 800000 edges per core
NSEG = 4                   # node-table segments (int16 index range)
S = N // NSEG              # 25000 rows per segment
NBUCKET = NSEG * NSEG      # 16 (src_seg, dst_seg) buckets
G = 5120                   # indices per dma_gather
CPG = G // P               # 50 dst columns per gather
B_PAD = 51200              # padded bucket size (10 chunks of G)
NCPB = B_PAD // G          # 8 chunks per bucket
TC = NBUCKET * NCPB        # 128 chunks per core
IW = G // 16               # idx columns per chunk (wrapped layout)
SCRATCH = 16384            # SWDGE descriptor-ring carveout bytes
SINGLE_PACKET = False      # one giant packet overflows the SWDGE ring; use
                           # multi-packet mode so the ucode reclaims space
NQUEUES = 4                # SWDGE queues: parallel Q7 descriptor generation


def build_nc():
    nc = bacc.Bacc(
        "TRN2",
        target_bir_lowering=False,
        debug=False,
        dynamic_dma_scratch_size=SCRATCH,
        num_swdge_queues=NQUEUES,
    )
    h = nc.dram_tensor("h", [N, D], mybir.dt.float32, kind="ExternalInput")
    sidx = nc.dram_tensor("sidx", [TC, P, IW], mybir.dt.int16, kind="ExternalInput")
    didx = nc.dram_tensor("didx", [TC, P, IW], mybir.dt.int16, kind="ExternalInput")
    out = nc.dram_tensor("out", [TC, P, CPG], mybir.dt.float32, kind="ExternalOutput")

    with tile.TileContext(nc) as tc:
        with (
            tc.tile_pool(name="idx", bufs=2) as idx_pool,
            tc.tile_pool(name="rows", bufs=4) as row_pool,
            tc.tile_pool(name="score", bufs=2) as score_pool,
        ):
            for c in range(TC):
                k = c // NCPB
                a, b = k // NSEG, k % NSEG
                idx_s = idx_pool.tile([P, IW], mybir.dt.int16, tag="s")
                idx_d = idx_pool.tile([P, IW], mybir.dt.int16, tag="d")
                nc.sync.dma_start(out=idx_s[:], in_=sidx[c])
                nc.sync.dma_start(out=idx_d[:], in_=didx[c])
                s_rows = row_pool.tile([P, CPG * D], mybir.dt.float32, tag="s")
                d_rows = row_pool.tile([P, CPG * D], mybir.dt.float32, tag="d")
                nc.gpsimd.dma_gather(
                    s_rows[:].rearrange("p (c d) -> p c d", d=D),
                    h[a * S : (a + 1) * S, :],
                    idx_s[:],
                    G,
                    G,
                    D,
                    single_packet=SINGLE_PACKET,
                    queue_num=(2 * c) % NQUEUES,
                )
                nc.gpsimd.dma_gather(
                    d_rows[:].rearrange("p (c d) -> p c d", d=D),
                    h[b * S : (b + 1) * S, :],
                    idx_d[:],
                    G,
                    G,
                    D,
                    single_packet=SINGLE_PACKET,
                    queue_num=(2 * c + 1) % NQUEUES,
                )
                nc.vector.tensor_tensor(
                    out=s_rows[:],
                    in0=s_rows[:],
                    in1=d_rows[:],
                    op=mybir.AluOpType.mult,
                )
                score = score_pool.tile([P, CPG], mybir.dt.float32, tag="sc")
                nc.vector.tensor_reduce(
                    out=score[:],
                    in_=s_rows[:].rearrange("p (c d) -> p c d", d=D),
                    axis=mybir.AxisListType.X,
                    op=mybir.AluOpType.add,
                )
                nc.sync.dma_start(out=out[c], in_=score[:])
    nc.compile()
    return nc


_NC_CACHE = None


def _get_nc():
    global _NC_CACHE
    if _NC_CACHE is None:
        _NC_CACHE = build_nc()
    return _NC_CACHE


def _prep_core(src_c, dst_c):
    """Bucket one core's edges. Returns (sidx, didx, pos, keep) where
    sidx/didx are the wrapped [TC, P, IW] int16 device index tensors, pos is
    each kept edge's flat position in the bucketed stream, keep the mask."""
    b = (src_c // S).astype(np.int32) * NSEG + (dst_c // S).astype(np.int32)
    # rank of each edge within its bucket, in original order
    rank = np.empty(E_PER, dtype=np.int64)
    for k in range(NBUCKET):
        m = b == k
        rank[m] = np.arange(m.sum(), dtype=np.int64)
    keep = rank < B_PAD
    pos = b.astype(np.int64) * B_PAD + rank  # valid where keep

    spad = np.zeros(NBUCKET * B_PAD, dtype=np.int16)
    dpad = np.zeros(NBUCKET * B_PAD, dtype=np.int16)
    kp = pos[keep]
    spad[kp] = (src_c[keep] % S).astype(np.int16)
    dpad[kp] = (dst_c[keep] % S).astype(np.int16)

    def wrap(arr):
        # [NBUCKET*B_PAD] -> [TC, G] -> wrapped [TC, 16, IW] -> tiled [TC, P, IW]
        a = arr.reshape(TC, IW, 16).transpose(0, 2, 1)
        return np.ascontiguousarray(np.tile(a, (1, P // 16, 1)))

    return wrap(spad), wrap(dpad), pos, keep


def run(inputs, trace=False, trace_kwargs=None):
    """Shard, execute on 8 cores, gather. Returns (scores[E] fp32, results)."""
    h = np.ascontiguousarray(np.asarray(inputs["h"], dtype=np.float32))
    src = np.ascontiguousarray(np.asarray(inputs["src"]).astype(np.int32))
    dst = np.ascontiguousarray(np.asarray(inputs["dst"]).astype(np.int32))
    assert h.shape == (N, D) and src.shape == (E,) and dst.shape == (E,)

    in_maps = []
    metas = []
    for i in range(M):
        sl = slice(i * E_PER, (i + 1) * E_PER)
        sidx, didx, pos, keep = _prep_core(src[sl], dst[sl])
        in_maps.append({"h": h, "sidx": sidx, "didx": didx})
        metas.append((pos, keep))

    try:
        res = run_bass_kernel_spmd(
            _get_nc(),
            in_maps,
            core_ids=list(range(M)),
            trace=trace,
            trace_kwargs=trace_kwargs or {},
        )
    except ModuleNotFoundError:
        # axon build without NTFF profiling hooks — run without trace
        res = run_bass_kernel_spmd(
            _get_nc(), in_maps, core_ids=list(range(M)), trace=False
        )

    scores = np.empty(E, dtype=np.float32)
    for i in range(M):
        sl = slice(i * E_PER, (i + 1) * E_PER)
        pos, keep = metas[i]
        out_arr = np.asarray(res.results[i]["out"], dtype=np.float32)
        # out_arr[c, p, j] is the score of bucketed position c*G + j*128 + p
        flat = out_arr.transpose(0, 2, 1).reshape(-1)
        sc = np.empty(E_PER, dtype=np.float32)
        sc[keep] = flat[pos[keep]]
        if not keep.all():  # host fallback for bucket-overflow edges
            ov = ~keep
            sc[ov] = np.einsum(
                "ed,ed->e", h[src[sl][ov]], h[dst[sl][ov]]
            ).astype(np.float32)
        scores[sl] = sc
    return scores, res


def kernel(**inputs) -> np.ndarray:
    return run(inputs)[0]

